# revision 23
# baseline (speedup 1.0000x reference)
"""Causal self-attention (B=4, T=2048, E=1024, H=16, D=64) on 8 TRN2 NeuronCores.

Sharding: core = b*2 + g  (data parallel over batch b in 0..3, tensor parallel
over head-halves g in 0..1; 8 local heads per core, column-split QKV /
row-split out projection). Host sums the two partial out-projections per batch
and adds b_out.

Device kernel (per core), fp16 matmul operands accumulating in fp32 PSUM:
  - blocks ordered t-block-outer (tb, pair): attention for (tb, p) emits
    scoresT chunks one ahead of exp (ACT) with av accumulation behind, causal
    diagonal via [128,128] triangle multiplies on DVE.
  - a budget-driven filler pump interleaves qkv-projection / v' / out-
    projection matmul slices between attention chunks so the PE never idles
    while ACT grinds exps; earliest-deadline-first with per-chunk PE/ACT
    cycle accounting.
  - startup: PE warm-spin matmuls (HAM un-throttle) + t-block-sliced x DMAs
    so the first scores chunk lands ~8us in.
  - softmax denominators ride the v' ones-columns; normalization multiplies
    PSUM directly by a broadcast reciprocal (DVE), h1 half relocated to
    partitions 64-127 by an SBUF-to-SBUF DMA.
"""
import numpy as np

B, T, E, H, D = 4, 2048, 1024, 16, 64
HL = H // 2           # local heads per core (8)
NP = HL // 2          # head pairs per core (4)
EL = HL * D           # local attn-out width (512)
VW = HL * (D + 1)     # v' width with ones columns (520)
NCORES = 8
NB = T // 512         # t-blocks (4)
NC = T // 128         # s-chunks (16)
NE = E // 128         # e-chunks (8)
HALF = VW // 2        # 260

_cache = {}

# pump cost constants (ns, warm clock)
_ACT_FULL = 1100.0
_PE_MM512 = 512 / 2.4 + 5
_COST_QK = 2 * _PE_MM512          # 2 MMs of N=512
_COST_V = 3 * (260 / 2.4 + 5)     # 3 MMs of N=260
_COST_PROJ = 2 * _PE_MM512


def _build_nc():
    import concourse.bacc as bacc
    import concourse.mybir as mybir
    from concourse.tile import TileContext

    F32 = mybir.dt.float32
    F16 = mybir.dt.float16
    EXP = mybir.ActivationFunctionType.Exp

    nc = bacc.Bacc(None, target_bir_lowering=False)
    xT = nc.dram_tensor("xT", [E, T], F16, kind="ExternalInput")
    wqk = nc.dram_tensor("wqk", [2 * NP, 128, NE, 128], F16, kind="ExternalInput")
    wv2d = nc.dram_tensor("wv2d", [2, 128, NE, HALF], F16, kind="ExternalInput")
    wo = nc.dram_tensor("wo", [EL, E], F16, kind="ExternalInput")
    rowsd = nc.dram_tensor("rowsd", [1, VW], F16, kind="ExternalInput")   # bv2
    bcold = nc.dram_tensor("bcold", [128, 2 * NP], F32, kind="ExternalInput")
    trid = nc.dram_tensor("trid", [128, 128], F32, kind="ExternalInput")
    y = nc.dram_tensor("y", [T, E], F16, kind="ExternalOutput")

    with TileContext(nc) as tc:
        with (
            tc.tile_pool(name="const", bufs=1) as cpool,
            tc.tile_pool(name="p_keep", bufs=1) as keep,
            tc.tile_pool(name="p_st", bufs=2) as st,
            tc.tile_pool(name="p_att", bufs=6) as att,
            tc.tile_pool(name="p_nrm", bufs=2) as nrm,
            tc.tile_pool(name="p_dr", bufs=2, space="DRAM") as drp,
            tc.tile_pool(name="psA", bufs=2, space="PSUM") as psA,
        ):
            # ---- long-lived tiles ----
            xt = [keep.tile([128, T], F16, name=f"xt{e}", tag=f"xt{e}") for e in range(NE)]
            wr = {}
            for p in range(NP):
                for i, nm in enumerate(("q", "k")):
                    wr[(p, nm)] = keep.tile([128, NE, 128], F16, name=f"w{nm}{p}", tag=f"w{nm}{p}")
            wv_r = [keep.tile([128, NE, HALF], F16, name=f"wv{h_}", tag=f"wv{h_}")
                    for h_ in range(2)]
            qt = [keep.tile([128, T], F16, name=f"qt{p}", tag=f"qt{p}") for p in range(NP)]
            kt = [keep.tile([128, T], F16, name=f"kt{p}", tag=f"kt{p}") for p in range(NP)]
            vt = [keep.tile([128, VW], F16, name=f"vt{t_}", tag=f"vt{t_}") for t_ in range(NC)]
            ao = [keep.tile([128, T], F16, name=f"ao{p}", tag=f"ao{p}") for p in range(NP)]
            wo_r = keep.tile([128, NP, E], F16, name="wo_r")
            tri_sb = cpool.tile([128, 128], F32, name="tri_sb")
            bcol = cpool.tile([128, 2 * NP], F32, name="bcol")
            ones_r = cpool.tile([1, 512], F16, name="ones_r")
            bv_r = cpool.tile([1, VW], F16, name="bv_r")
            warm16 = cpool.tile([128, 128], F16, name="warm16")
            wout = cpool.tile([1, 16], F32, name="wout")

            # ---- warmup: ACT exp table + PE HAM spin ----
            nc.vector.memset(warm16, 0.5)
            nc.vector.memset(ones_r, 1.0)
            nc.scalar.activation(wout, warm16[0:1, 0:16], EXP, scale=0.125)
            wps = psA.tile([128, 512], F32, name="wps", tag="f")
            for _ in range(40):
                nc.tensor.matmul(wps[:, 0:128], warm16, warm16, start=True, stop=True)

            # ---- DMA priority order (two queues) ----
            # sync queue: pair weights + x tb0/tb1/tb2
            # gpsimd queue: v weights + x tb3 + wo
            nc.sync.dma_start(out=wr[(0, "q")], in_=wqk[0])
            nc.sync.dma_start(out=wr[(0, "k")], in_=wqk[1])
            nc.sync.dma_start(out=tri_sb, in_=trid[:, :])
            nc.sync.dma_start(out=bcol, in_=bcold[:, :])
            nc.sync.dma_start(out=bv_r, in_=rowsd[:, :])
            for e in range(NE):
                nc.sync.dma_start(out=xt[e][:, 0:512], in_=xT[e * 128:(e + 1) * 128, 0:512])
            nc.gpsimd.dma_start(out=wv_r[0], in_=wv2d[0])
            nc.gpsimd.dma_start(out=wv_r[1], in_=wv2d[1])
            for i, nm in ((1, "q"), (1, "k")):
                nc.sync.dma_start(out=wr[(i, nm)], in_=wqk[2 * i + (0 if nm == "q" else 1)])
            for e in range(NE):
                nc.gpsimd.dma_start(out=xt[e][:, 512:1024], in_=xT[e * 128:(e + 1) * 128, 512:1024])
            for i in (2, 3):
                for k, nm in enumerate(("q", "k")):
                    nc.sync.dma_start(out=wr[(i, nm)], in_=wqk[2 * i + k])
            for e in range(NE):
                nc.sync.dma_start(out=xt[e][:, 1024:1536], in_=xT[e * 128:(e + 1) * 128, 1024:1536])
            for e in range(NE):
                nc.gpsimd.dma_start(out=xt[e][:, 1536:2048], in_=xT[e * 128:(e + 1) * 128, 1536:2048])
            for p in range(NP):
                nc.gpsimd.dma_start(out=wo_r[:, p, :], in_=wo[p * 128:(p + 1) * 128, :])

            # ---- filler unit machinery ----
            units = []   # sorted list of (due, seq, cost, emit_fn)
            seq_ctr = [0]
            state_tail = [False]

            def add_unit(due, cost, fn):
                units.append((due, seq_ctr[0], cost, fn))
                seq_ctr[0] += 1

            def add_qk_units(p, nm, tb, due):
                st_ = {}
                col = 2 * p + (0 if nm == "q" else 1)
                dst = (qt if nm == "q" else kt)[p]

                def mk(i):
                    def emit():
                        if i == 0:
                            st_["ps"] = psA.tile([128, 512], F32,
                                                 name=f"fqk{p}{nm}{tb}", tag="f")
                        ps = st_["ps"]
                        for e in (2 * i, 2 * i + 1):
                            nc.tensor.matmul(
                                ps, wr[(p, nm)][:, e, :], xt[e][:, tb * 512:(tb + 1) * 512],
                                start=(e == 0), stop=(e == NE - 1))
                        if i == 3:
                            nc.vector.tensor_scalar_add(
                                dst[:, tb * 512:(tb + 1) * 512], ps, bcol[:, col:col + 1])
                    return emit
                for i in range(4):
                    add_unit(due, _COST_QK, mk(i))

            def add_v_units(tt, h, due):
                st_ = {}

                def mk(i):
                    def emit():
                        if i == 0:
                            st_["ps"] = psA.tile([128, HALF], F32,
                                                 name=f"fv{tt}{h}", tag="f")
                        ps = st_["ps"]
                        for k in range(3 * i, 3 * i + 3):
                            if k < NE:
                                nc.tensor.matmul(
                                    ps, xt[k][:, tt * 128:(tt + 1) * 128], wv_r[h][:, k, :],
                                    start=(k == 0), stop=False)
                            else:
                                nc.tensor.matmul(
                                    ps, ones_r[:, 0:128], bv_r[:, h * HALF:(h + 1) * HALF],
                                    start=False, stop=True)
                        if i == 2:
                            nc.vector.tensor_copy(vt[tt][:, h * HALF:(h + 1) * HALF], ps)
                    return emit
                for i in range(3):
                    add_unit(due, _COST_V, mk(i))

            proj_ctr = [0]

            def add_proj_units(tt, eb):
                st_ = {}

                def mk(i):
                    def emit():
                        if i == 0:
                            st_["ps"] = psA.tile([128, 512], F32,
                                                 name=f"fp{tt}{eb}", tag="f")
                        ps = st_["ps"]
                        for pp in (2 * i, 2 * i + 1):
                            nc.tensor.matmul(
                                ps, ao[pp][:, tt * 128:(tt + 1) * 128],
                                wo_r[:, pp, eb * 512:(eb + 1) * 512],
                                start=(pp == 0), stop=(pp == NP - 1))
                        if i == 1:
                            ys = st.tile([128, 512], F16, name=f"ys{tt}{eb}",
                                         tag="ys", bufs=4)
                            proj_ctr[0] += 1
                            if state_tail[0] and proj_ctr[0] % 2 == 0:
                                nc.scalar.copy(ys, ps)
                            else:
                                nc.vector.tensor_copy(ys, ps)
                            nc.sync.dma_start(
                                out=y[tt * 128:(tt + 1) * 128, eb * 512:(eb + 1) * 512],
                                in_=ys)
                    return emit
                for i in range(2):
                    add_unit((999, proj_ctr[0]), _COST_PROJ, mk(i))

            # deadlines: qk(p,nm,tb) drained at last chunk of the previous
            # block (just before that block's cross-block scores lookahead);
            # v(tt,h) drained at chunk tt of block (tt//4, p=0).
            blocks = [(tb, p) for tb in range(NB) for p in range(NP)]
            nch_of = lambda tb: 4 * (tb + 1)
            for tb in range(NB):
                for p in range(NP):
                    bi = tb * NP + p
                    if bi == 0:
                        due = (-1, 0)
                    else:
                        ptb, pp_ = blocks[bi - 1]
                        due = (bi - 1, nch_of(ptb) - 1)
                    for nm in ("q", "k"):
                        add_qk_units(p, nm, tb, due)
            for tt in range(NC):
                for h in range(2):
                    add_v_units(tt, h, ((tt // 4) * NP, tt))
            units.sort(key=lambda u: (u[0], u[1]))

            debt = [0.0]

            def drain_due(key):
                while units and units[0][0] <= key:
                    _, _, cost, fn = units.pop(0)
                    fn()
                    debt[0] = max(debt[0] - cost, -2500.0)

            def pump():
                while debt[0] > 0 and units:
                    _, _, cost, fn = units.pop(0)
                    fn()
                    debt[0] -= cost

            # ---- attention emitters ----
            def emit_scores(p, tb, c):
                j = c - 4 * tb
                lo = 128 * j if j >= 0 else 0
                sp = psA.tile([128, 1024], F32, name=f"s_{p}_{tb}_{c}", tag="s")
                nc.tensor.matmul(
                    sp[:, lo:512], kt[p][0:64, c * 128:(c + 1) * 128],
                    qt[p][0:64, tb * 512 + lo:(tb + 1) * 512],
                    start=True, stop=True, tile_position=(0, 0),
                )
                nc.tensor.matmul(
                    sp[:, 512 + lo:1024], kt[p][64:128, c * 128:(c + 1) * 128],
                    qt[p][64:128, tb * 512 + lo:(tb + 1) * 512],
                    start=True, stop=True, tile_position=(64, 0),
                )
                return sp, lo, j

            def emit_exp(p, tb, c, sc, av_queue):
                sp, lo, j = sc
                ep = att.tile([128, 1024], F16, name=f"e_{p}_{tb}_{c}", tag="ep")
                if j < 0:
                    nc.scalar.activation(ep, sp, EXP, scale=0.125)
                    act_cost = _ACT_FULL
                else:
                    spv = sp[:, :].rearrange("q (h t) -> q h t", h=2)
                    epv = ep[:, :].rearrange("q (h t) -> q h t", h=2)
                    nc.scalar.activation(epv[:, :, lo:512], spv[:, :, lo:512],
                                         EXP, scale=0.125)
                    for h in range(2):
                        nc.gpsimd.tensor_mul(
                            epv[:, h, lo:lo + 128], epv[:, h, lo:lo + 128], tri_sb)
                    act_cost = 2 * (512 - lo) * 0.833 + 250
                av_queue.append((c, ep, lo))
                pe_cost = 3 * (512 - lo) / 2.4 + 24
                return act_cost - pe_cost

            # ---- normalization (three stages, spread across the next block
            # so neither the DVE FIFO head nor the av psum slots block) ----
            def emit_norm1(tb, p, av0, av1):
                # d-rows out of PSUM (DVE; deps just met by the last av MM),
                # then bounce through DRAM for the partition broadcast
                dsb = nrm.tile([65, 1024], F32, name=f"dsb_{tb}_{p}", tag="dsb")
                nc.vector.tensor_copy(dsb[64:65, 0:512], av0[64:65, :])
                nc.vector.tensor_copy(dsb[64:65, 512:1024], av1[64:65, :])
                ds = drp.tile([1, 1024], F32, name=f"ds_{tb}_{p}", tag="ds")
                nc.gpsimd.dma_start(out=ds, in_=dsb[64:65, :])
                return ds

            def emit_norm2a(st_):
                tb, p, av0, av1, ds = st_
                bc = nrm.tile([64, 1024], F32, name=f"bc_{tb}_{p}", tag="bc")
                nc.sync.dma_start(out=bc[:, 0:512],
                                  in_=ds[0:1, 0:512].partition_broadcast(64))
                nc.sync.dma_start(out=bc[:, 512:1024],
                                  in_=ds[0:1, 512:1024].partition_broadcast(64))
                return bc

            def emit_norm2b(st_, bc):
                tb, p, av0, av1, ds = st_
                rc = nrm.tile([64, 1024], F32, name=f"rc_{tb}_{p}", tag="rc")
                nc.vector.reciprocal_approx_fast(out=rc, in_=bc)
                nc.vector.tensor_mul(ao[p][0:64, tb * 512:(tb + 1) * 512],
                                     av0[0:64, :], rc[:, 0:512])
                aot = nrm.tile([64, 512], F16, name=f"aot_{tb}_{p}", tag="aot")
                nc.vector.tensor_mul(aot, av1[0:64, :], rc[:, 512:1024])
                nc.sync.dma_start(out=ao[p][64:128, tb * 512:(tb + 1) * 512], in_=aot)
                if p == NP - 1:
                    for tt in range(4 * tb, 4 * tb + 4):
                        for eb in range(2):
                            if tb == 2 and tt >= 4 * tb + 2:
                                reserved.append((tt, eb))   # tail warm-keeper
                            else:
                                add_proj_units(tt, eb)

            reserved = []

            def emit_proj_group_direct(tt, eb):
                ps = psA.tile([128, 512], F32, name=f"rp{tt}{eb}", tag="f")
                for pp in range(NP):
                    nc.tensor.matmul(
                        ps, ao[pp][:, tt * 128:(tt + 1) * 128],
                        wo_r[:, pp, eb * 512:(eb + 1) * 512],
                        start=(pp == 0), stop=(pp == NP - 1))
                ys = st.tile([128, 512], F16, name=f"ys{tt}{eb}", tag="ys", bufs=4)
                proj_ctr[0] += 1
                if proj_ctr[0] % 2 == 0:
                    nc.scalar.copy(ys, ps)
                else:
                    nc.vector.tensor_copy(ys, ps)
                dma_eng = nc.gpsimd if proj_ctr[0] % 2 == 0 else nc.sync
                dma_eng.dma_start(
                    out=y[tt * 128:(tt + 1) * 128, eb * 512:(eb + 1) * 512], in_=ys)

            # ---- main loop ----
            pre_sc = None
            prev_norm = None
            prev_bc = None
            for bi, (tb, p) in enumerate(blocks):
                nch = nch_of(tb)
                drain_due((bi - 1, 99))
                av0 = av1 = None
                av_queue = []
                sc = pre_sc if pre_sc is not None else emit_scores(p, tb, 0)
                pre_sc = None

                def flush_avs(limit):
                    nonlocal av0, av1
                    while av_queue and len(av_queue) > limit:
                        if av0 is None:
                            av0 = psA.tile([65, 512], F32, name=f"av0_{tb}_{p}", tag="av")
                            av1 = psA.tile([65, 512], F32, name=f"av1_{tb}_{p}", tag="av")
                        cc, ep, lo = av_queue.pop(0)
                        for h, av in ((0, av0), (1, av1)):
                            vcol = 65 * (2 * p + h)
                            nc.tensor.matmul(
                                av[:, lo:512], vt[cc][:, vcol:vcol + 65],
                                ep[:, 512 * h + lo:512 * h + 512],
                                start=(cc == 0), stop=(cc == nch - 1))

                av_start = 3 if nch <= 4 else 4
                for c in range(nch):
                    if c + 1 < nch:
                        sc_next = emit_scores(p, tb, c + 1)
                        drain_due((bi, c))
                    else:
                        # cross-block lookahead: its qt/kt writes must be
                        # emitted first, so drain before the scores here
                        drain_due((bi, c))
                        sc_next = None
                        if bi + 1 < len(blocks):
                            ntb, np_ = blocks[bi + 1]
                            pre_sc = emit_scores(np_, ntb, 0)
                    if c == 0 and prev_norm is not None:
                        prev_bc = emit_norm2a(prev_norm)
                    d = emit_exp(p, tb, c, sc, av_queue)
                    if c == 1 and prev_norm is not None:
                        emit_norm2b(prev_norm, prev_bc)
                        prev_norm = None
                    debt[0] = min(debt[0] + d, 3500.0)
                    pump()
                    if c >= av_start:
                        flush_avs(1)
                    sc = sc_next
                flush_avs(0)
                ds = emit_norm1(tb, p, av0, av1)
                prev_norm = (tb, p, av0, av1, ds)

            # ---- tail ----
            state_tail[0] = True
            prev_bc = emit_norm2a(prev_norm)
            for tt, eb in reserved[:2]:
                emit_proj_group_direct(tt, eb)
            emit_norm2b(prev_norm, prev_bc)
            for tt, eb in reserved[2:]:
                emit_proj_group_direct(tt, eb)
            while units:
                _, _, _, fn = units.pop(0)
                fn()

    nc.compile()
    return nc


def get_nc():
    if "nc" not in _cache:
        _cache["nc"] = _build_nc()
    return _cache["nc"]


def make_in_maps(x, w_qkv, b_qkv, w_out, b_out):
    """Per-core input dicts. Core = b*2 + g."""
    x = np.asarray(x, dtype=np.float32)
    w_qkv = np.asarray(w_qkv, dtype=np.float32)
    b_qkv = np.asarray(b_qkv, dtype=np.float32)
    w_out = np.asarray(w_out, dtype=np.float32)

    wq_full, wk_full, wv_full = w_qkv[:, 0:E], w_qkv[:, E:2 * E], w_qkv[:, 2 * E:3 * E]
    bq_full, bk_full, bv_full = b_qkv[0:E], b_qkv[E:2 * E], b_qkv[2 * E:3 * E]

    idx = np.arange(128)
    tri = (idx[:, None] <= idx[None, :]).astype(np.float32)  # tri[s,t]=1 iff s<=t

    in_maps = []
    for core in range(NCORES):
        b, g = core // 2, core % 2
        h0 = g * HL
        cols = slice(h0 * D, (h0 + HL) * D)
        wq_l = wq_full[:, cols]
        wk_l = wk_full[:, cols]
        wv_l = wv_full[:, cols]
        bq_l = bq_full[cols]
        bk_l = bk_full[cols]
        bv_l = bv_full[cols]

        wqk_s = np.empty((2 * NP, 128, NE, 128), dtype=np.float16)
        for p in range(NP):
            wqk_s[2 * p] = wq_l[:, p * 128:(p + 1) * 128].reshape(NE, 128, 128).transpose(1, 0, 2)
            wqk_s[2 * p + 1] = wk_l[:, p * 128:(p + 1) * 128].reshape(NE, 128, 128).transpose(1, 0, 2)

        wv2 = np.zeros((E, VW), dtype=np.float16)
        bv2 = np.zeros((1, VW), dtype=np.float16)
        for h in range(HL):
            wv2[:, h * 65:h * 65 + 64] = wv_l[:, h * 64:(h + 1) * 64].astype(np.float16)
            bv2[0, h * 65:h * 65 + 64] = bv_l[h * 64:(h + 1) * 64].astype(np.float16)
            bv2[0, h * 65 + 64] = 1.0

        bcol = np.zeros((128, 2 * NP), dtype=np.float32)
        for p in range(NP):
            bcol[:, 2 * p] = bq_l[p * 128:(p + 1) * 128]
            bcol[:, 2 * p + 1] = bk_l[p * 128:(p + 1) * 128]

        wv2d = wv2.reshape(NE, 128, 2, VW // 2).transpose(2, 1, 0, 3)
        in_maps.append({
            "xT": np.ascontiguousarray(x[b].T.astype(np.float16)),
            "wqk": np.ascontiguousarray(wqk_s),
            "wv2d": np.ascontiguousarray(wv2d),
            "wo": np.ascontiguousarray(w_out[g * EL:(g + 1) * EL, :]).astype(np.float16),
            "rowsd": bv2,
            "bcold": bcol,
            "trid": tri,
        })
    return in_maps


def gather_output(results, b_out):
    out = np.empty((B, T, E), dtype=np.float32)
    for b in range(B):
        out[b] = (results[2 * b]["y"].astype(np.float32)
                  + results[2 * b + 1]["y"].astype(np.float32) + b_out[None, :])
    return out


def kernel(x, w_qkv, b_qkv, w_out, b_out):
    from concourse.bass_utils import run_bass_kernel_spmd

    nc = get_nc()
    in_maps = make_in_maps(x, w_qkv, b_qkv, w_out, b_out)
    r = run_bass_kernel_spmd(nc, in_maps, core_ids=list(range(NCORES)))
    return gather_output(r.results, np.asarray(b_out, dtype=np.float32))


# revision 24
# speedup vs baseline: 1.1800x; 1.1800x over previous
"""Causal self-attention (B=4, T=2048, E=1024, H=16, D=64) on 8 TRN2 NeuronCores.

Sharding: core = b*2 + g  (data parallel over batch b in 0..3, tensor parallel
over head-halves g in 0..1; 8 local heads per core, column-split QKV /
row-split out projection). Host sums the two partial out-projections per batch
and adds b_out.

Device kernel (per core), fp16 matmul operands accumulating in fp32 PSUM:
  - blocks ordered t-block-outer (tb, pair): attention for (tb, p) emits
    scoresT chunks one ahead of exp (ACT) with av accumulation behind, causal
    diagonal via [128,128] triangle multiplies on DVE.
  - a budget-driven filler pump interleaves qkv-projection / v' / out-
    projection matmul slices between attention chunks so the PE never idles
    while ACT grinds exps; earliest-deadline-first with per-chunk PE/ACT
    cycle accounting.
  - startup: PE warm-spin matmuls (HAM un-throttle) + t-block-sliced x DMAs
    so the first scores chunk lands ~8us in.
  - softmax denominators ride the v' ones-columns; normalization multiplies
    PSUM directly by a broadcast reciprocal (DVE), h1 half relocated to
    partitions 64-127 by an SBUF-to-SBUF DMA.
"""
import numpy as np

B, T, E, H, D = 4, 2048, 1024, 16, 64
HL = H // 2           # local heads per core (8)
NP = HL // 2          # head pairs per core (4)
EL = HL * D           # local attn-out width (512)
VW = HL * (D + 1)     # v' width with ones columns (520)
NCORES = 8
NB = T // 512         # t-blocks (4)
NC = T // 128         # s-chunks (16)
NE = E // 128         # e-chunks (8)
HALF = VW // 2        # 260

_cache = {}

# pump cost constants (ns, warm clock)
_ACT_FULL = 1100.0
_PE_MM512 = 512 / 2.4 + 5
_COST_QK = 2 * _PE_MM512          # 2 MMs of N=512
_COST_V = 3 * (260 / 2.4 + 5)     # 3 MMs of N=260
_COST_PROJ = 2 * _PE_MM512


def _build_nc():
    import concourse.bacc as bacc
    import concourse.mybir as mybir
    from concourse.tile import TileContext

    F32 = mybir.dt.float32
    F16 = mybir.dt.float16
    EXP = mybir.ActivationFunctionType.Exp

    nc = bacc.Bacc(None, target_bir_lowering=False)
    xT = nc.dram_tensor("xT", [E, T], F16, kind="ExternalInput")
    wqk = nc.dram_tensor("wqk", [2 * NP, 128, NE, 128], F16, kind="ExternalInput")
    wv2d = nc.dram_tensor("wv2d", [2, 128, NE, HALF], F16, kind="ExternalInput")
    wo = nc.dram_tensor("wo", [EL, E], F16, kind="ExternalInput")
    rowsd = nc.dram_tensor("rowsd", [1, VW], F16, kind="ExternalInput")   # bv2
    bcold = nc.dram_tensor("bcold", [128, 2 * NP], F32, kind="ExternalInput")
    trid = nc.dram_tensor("trid", [128, 128], F32, kind="ExternalInput")
    y = nc.dram_tensor("y", [T, E], F16, kind="ExternalOutput")

    with TileContext(nc) as tc:
        with (
            tc.tile_pool(name="const", bufs=1) as cpool,
            tc.tile_pool(name="p_keep", bufs=1) as keep,
            tc.tile_pool(name="p_st", bufs=2) as st,
            tc.tile_pool(name="p_att", bufs=6) as att,
            tc.tile_pool(name="p_nrm", bufs=2) as nrm,
            tc.tile_pool(name="p_dr", bufs=2, space="DRAM") as drp,
            tc.tile_pool(name="psA", bufs=2, space="PSUM") as psA,
        ):
            # ---- long-lived tiles ----
            xt = [keep.tile([128, T], F16, name=f"xt{e}", tag=f"xt{e}") for e in range(NE)]
            wr = {}
            for p in range(NP):
                for i, nm in enumerate(("q", "k")):
                    wr[(p, nm)] = keep.tile([128, NE, 128], F16, name=f"w{nm}{p}", tag=f"w{nm}{p}")
            wv_r = [keep.tile([128, NE, HALF], F16, name=f"wv{h_}", tag=f"wv{h_}")
                    for h_ in range(2)]
            qt = [keep.tile([128, T], F16, name=f"qt{p}", tag=f"qt{p}") for p in range(NP)]
            kt = [keep.tile([128, T], F16, name=f"kt{p}", tag=f"kt{p}") for p in range(NP)]
            vt = [keep.tile([128, VW], F16, name=f"vt{t_}", tag=f"vt{t_}") for t_ in range(NC)]
            ao = [keep.tile([128, T], F16, name=f"ao{p}", tag=f"ao{p}") for p in range(NP)]
            wo_r = keep.tile([128, NP, E], F16, name="wo_r")
            tri_sb = cpool.tile([128, 128], F32, name="tri_sb")
            bcol = cpool.tile([128, 2 * NP], F32, name="bcol")
            ones_r = cpool.tile([1, 512], F16, name="ones_r")
            bv_r = cpool.tile([1, VW], F16, name="bv_r")
            warm16 = cpool.tile([128, 128], F16, name="warm16")
            wout = cpool.tile([1, 16], F32, name="wout")

            # ---- warmup: ACT exp table + PE HAM spin ----
            nc.vector.memset(warm16, 0.5)
            nc.vector.memset(ones_r, 1.0)
            nc.scalar.activation(wout, warm16[0:1, 0:16], EXP, scale=0.125)
            wps = psA.tile([128, 512], F32, name="wps", tag="f")
            for _ in range(40):
                nc.tensor.matmul(wps[:, 0:128], warm16, warm16, start=True, stop=True)

            # ---- DMA priority order (two queues) ----
            # sync queue: pair weights + x tb0/tb1/tb2
            # gpsimd queue: v weights + x tb3 + wo
            nc.sync.dma_start(out=wr[(0, "q")], in_=wqk[0])
            nc.sync.dma_start(out=wr[(0, "k")], in_=wqk[1])
            nc.sync.dma_start(out=tri_sb, in_=trid[:, :])
            nc.sync.dma_start(out=bcol, in_=bcold[:, :])
            nc.sync.dma_start(out=bv_r, in_=rowsd[:, :])
            for e in range(NE):
                nc.sync.dma_start(out=xt[e][:, 0:512], in_=xT[e * 128:(e + 1) * 128, 0:512])
            nc.gpsimd.dma_start(out=wv_r[0], in_=wv2d[0])
            nc.gpsimd.dma_start(out=wv_r[1], in_=wv2d[1])
            for i, nm in ((1, "q"), (1, "k")):
                nc.sync.dma_start(out=wr[(i, nm)], in_=wqk[2 * i + (0 if nm == "q" else 1)])
            for e in range(NE):
                nc.gpsimd.dma_start(out=xt[e][:, 512:1024], in_=xT[e * 128:(e + 1) * 128, 512:1024])
            for i in (2, 3):
                for k, nm in enumerate(("q", "k")):
                    nc.sync.dma_start(out=wr[(i, nm)], in_=wqk[2 * i + k])
            for e in range(NE):
                nc.sync.dma_start(out=xt[e][:, 1024:1536], in_=xT[e * 128:(e + 1) * 128, 1024:1536])
            for e in range(NE):
                nc.gpsimd.dma_start(out=xt[e][:, 1536:2048], in_=xT[e * 128:(e + 1) * 128, 1536:2048])
            for p in range(NP):
                nc.gpsimd.dma_start(out=wo_r[:, p, :], in_=wo[p * 128:(p + 1) * 128, :])

            # ---- filler unit machinery ----
            units = []   # sorted list of (due, seq, cost, emit_fn)
            seq_ctr = [0]
            state_tail = [False]

            def add_unit(due, cost, fn):
                units.append((due, seq_ctr[0], cost, fn))
                seq_ctr[0] += 1

            def add_qk_units(p, nm, tb, due):
                st_ = {}
                col = 2 * p + (0 if nm == "q" else 1)
                dst = (qt if nm == "q" else kt)[p]

                def mk(i):
                    def emit():
                        if i == 0:
                            st_["ps"] = psA.tile([128, 512], F32,
                                                 name=f"fqk{p}{nm}{tb}", tag="f")
                        ps = st_["ps"]
                        for e in (2 * i, 2 * i + 1):
                            nc.tensor.matmul(
                                ps, wr[(p, nm)][:, e, :], xt[e][:, tb * 512:(tb + 1) * 512],
                                start=(e == 0), stop=(e == NE - 1))
                        if i == 3:
                            nc.vector.tensor_scalar_add(
                                dst[:, tb * 512:(tb + 1) * 512], ps, bcol[:, col:col + 1])
                    return emit
                for i in range(4):
                    add_unit(due, _COST_QK, mk(i))

            def add_v_units(tt, h, due):
                st_ = {}

                def mk(i):
                    def emit():
                        if i == 0:
                            st_["ps"] = psA.tile([128, HALF], F32,
                                                 name=f"fv{tt}{h}", tag="f")
                        ps = st_["ps"]
                        for k in range(3 * i, 3 * i + 3):
                            if k < NE:
                                nc.tensor.matmul(
                                    ps, xt[k][:, tt * 128:(tt + 1) * 128], wv_r[h][:, k, :],
                                    start=(k == 0), stop=False)
                            else:
                                nc.tensor.matmul(
                                    ps, ones_r[:, 0:128], bv_r[:, h * HALF:(h + 1) * HALF],
                                    start=False, stop=True)
                        if i == 2:
                            nc.vector.tensor_copy(vt[tt][:, h * HALF:(h + 1) * HALF], ps)
                    return emit
                for i in range(3):
                    add_unit(due, _COST_V, mk(i))

            proj_ctr = [0]

            def add_proj_units(tt, eb):
                st_ = {}

                def mk(i):
                    def emit():
                        if i == 0:
                            st_["ps"] = psA.tile([128, 512], F32,
                                                 name=f"fp{tt}{eb}", tag="f")
                        ps = st_["ps"]
                        for pp in (2 * i, 2 * i + 1):
                            nc.tensor.matmul(
                                ps, ao[pp][:, tt * 128:(tt + 1) * 128],
                                wo_r[:, pp, eb * 512:(eb + 1) * 512],
                                start=(pp == 0), stop=(pp == NP - 1))
                        if i == 1:
                            ys = st.tile([128, 512], F16, name=f"ys{tt}{eb}",
                                         tag="ys", bufs=4)
                            proj_ctr[0] += 1
                            if state_tail[0] and proj_ctr[0] % 2 == 0:
                                nc.scalar.copy(ys, ps)
                            else:
                                nc.vector.tensor_copy(ys, ps)
                            nc.sync.dma_start(
                                out=y[tt * 128:(tt + 1) * 128, eb * 512:(eb + 1) * 512],
                                in_=ys)
                    return emit
                for i in range(2):
                    add_unit((999, proj_ctr[0]), _COST_PROJ, mk(i))

            # deadlines: qk(p,nm,tb) drained at last chunk of the previous
            # block (just before that block's cross-block scores lookahead);
            # v(tt,h) drained at chunk tt of block (tt//4, p=0).
            blocks = [(tb, p) for tb in range(NB) for p in range(NP)]
            nch_of = lambda tb: 4 * (tb + 1)
            for tb in range(NB):
                for p in range(NP):
                    bi = tb * NP + p
                    if bi == 0:
                        due = (-1, 0)
                    else:
                        ptb, pp_ = blocks[bi - 1]
                        due = (bi - 1, nch_of(ptb) - 1)
                    for nm in ("q", "k"):
                        add_qk_units(p, nm, tb, due)
            for tt in range(NC):
                for h in range(2):
                    add_v_units(tt, h, ((tt // 4) * NP, tt))
            units.sort(key=lambda u: (u[0], u[1]))

            debt = [0.0]

            def drain_due(key):
                while units and units[0][0] <= key:
                    _, _, cost, fn = units.pop(0)
                    fn()
                    debt[0] = max(debt[0] - cost, -2500.0)

            def pump():
                while debt[0] > 0 and units:
                    _, _, cost, fn = units.pop(0)
                    fn()
                    debt[0] -= cost

            # ---- attention emitters ----
            def emit_scores(p, tb, c):
                j = c - 4 * tb
                lo = 128 * j if j >= 0 else 0
                sp = psA.tile([128, 1024], F32, name=f"s_{p}_{tb}_{c}", tag="s")
                nc.tensor.matmul(
                    sp[:, lo:512], kt[p][0:64, c * 128:(c + 1) * 128],
                    qt[p][0:64, tb * 512 + lo:(tb + 1) * 512],
                    start=True, stop=True, tile_position=(0, 0),
                )
                nc.tensor.matmul(
                    sp[:, 512 + lo:1024], kt[p][64:128, c * 128:(c + 1) * 128],
                    qt[p][64:128, tb * 512 + lo:(tb + 1) * 512],
                    start=True, stop=True, tile_position=(64, 0),
                )
                return sp, lo, j

            def emit_exp(p, tb, c, sc, av_queue):
                sp, lo, j = sc
                ep = att.tile([128, 1024], F16, name=f"e_{p}_{tb}_{c}", tag="ep")
                if j < 0:
                    nc.scalar.activation(ep, sp, EXP, scale=0.125)
                    act_cost = _ACT_FULL
                else:
                    spv = sp[:, :].rearrange("q (h t) -> q h t", h=2)
                    epv = ep[:, :].rearrange("q (h t) -> q h t", h=2)
                    nc.scalar.activation(epv[:, :, lo:512], spv[:, :, lo:512],
                                         EXP, scale=0.125)
                    for h in range(2):
                        nc.gpsimd.tensor_mul(
                            epv[:, h, lo:lo + 128], epv[:, h, lo:lo + 128], tri_sb)
                    act_cost = 2 * (512 - lo) * 0.833 + 250
                av_queue.append((c, ep, lo))
                pe_cost = 3 * (512 - lo) / 2.4 + 24
                return act_cost - pe_cost

            # ---- normalization (three stages, spread across the next block
            # so neither the DVE FIFO head nor the av psum slots block) ----
            def emit_norm1(tb, p, av0, av1):
                # d-rows out of PSUM (DVE; deps just met by the last av MM),
                # then bounce through DRAM for the partition broadcast
                dsb = nrm.tile([65, 1024], F32, name=f"dsb_{tb}_{p}", tag="dsb")
                nc.vector.tensor_copy(dsb[64:65, 0:512], av0[64:65, :])
                nc.vector.tensor_copy(dsb[64:65, 512:1024], av1[64:65, :])
                ds = drp.tile([1, 1024], F32, name=f"ds_{tb}_{p}", tag="ds")
                nc.gpsimd.dma_start(out=ds, in_=dsb[64:65, :])
                return ds

            def emit_norm2a(st_):
                tb, p, av0, av1, ds = st_
                bc = nrm.tile([64, 1024], F32, name=f"bc_{tb}_{p}", tag="bc")
                nc.sync.dma_start(out=bc[:, 0:512],
                                  in_=ds[0:1, 0:512].partition_broadcast(64))
                nc.sync.dma_start(out=bc[:, 512:1024],
                                  in_=ds[0:1, 512:1024].partition_broadcast(64))
                return bc

            def emit_norm2b(st_, bc):
                tb, p, av0, av1, ds = st_
                rc = nrm.tile([64, 1024], F32, name=f"rc_{tb}_{p}", tag="rc")
                nc.vector.reciprocal_approx_fast(out=rc, in_=bc)
                nc.vector.tensor_mul(ao[p][0:64, tb * 512:(tb + 1) * 512],
                                     av0[0:64, :], rc[:, 0:512])
                aot = nrm.tile([64, 512], F16, name=f"aot_{tb}_{p}", tag="aot")
                nc.vector.tensor_mul(aot, av1[0:64, :], rc[:, 512:1024])
                nc.sync.dma_start(out=ao[p][64:128, tb * 512:(tb + 1) * 512], in_=aot)
                if p == NP - 1:
                    for tt in range(4 * tb, 4 * tb + 4):
                        for eb in range(2):
                            if tb == 2 and tt >= 4 * tb + 2:
                                reserved.append((tt, eb))   # tail warm-keeper
                            else:
                                add_proj_units(tt, eb)

            reserved = []

            def emit_proj_group_direct(tt, eb):
                ps = psA.tile([128, 512], F32, name=f"rp{tt}{eb}", tag="f")
                for pp in range(NP):
                    nc.tensor.matmul(
                        ps, ao[pp][:, tt * 128:(tt + 1) * 128],
                        wo_r[:, pp, eb * 512:(eb + 1) * 512],
                        start=(pp == 0), stop=(pp == NP - 1))
                ys = st.tile([128, 512], F16, name=f"ys{tt}{eb}", tag="ys", bufs=4)
                proj_ctr[0] += 1
                if proj_ctr[0] % 2 == 0:
                    nc.scalar.copy(ys, ps)
                else:
                    nc.vector.tensor_copy(ys, ps)
                dma_eng = nc.gpsimd if proj_ctr[0] % 2 == 0 else nc.sync
                dma_eng.dma_start(
                    out=y[tt * 128:(tt + 1) * 128, eb * 512:(eb + 1) * 512], in_=ys)

            # ---- main loop ----
            pre_sc = None
            prev_norm = None
            prev_bc = None
            for bi, (tb, p) in enumerate(blocks):
                nch = nch_of(tb)
                drain_due((bi - 1, 99))
                av0 = av1 = None
                av_queue = []
                sc = pre_sc if pre_sc is not None else emit_scores(p, tb, 0)
                pre_sc = None

                def flush_avs(limit):
                    nonlocal av0, av1
                    while av_queue and len(av_queue) > limit:
                        if av0 is None:
                            av0 = psA.tile([65, 512], F32, name=f"av0_{tb}_{p}", tag="av")
                            av1 = psA.tile([65, 512], F32, name=f"av1_{tb}_{p}", tag="av")
                        cc, ep, lo = av_queue.pop(0)
                        for h, av in ((0, av0), (1, av1)):
                            vcol = 65 * (2 * p + h)
                            nc.tensor.matmul(
                                av[:, lo:512], vt[cc][:, vcol:vcol + 65],
                                ep[:, 512 * h + lo:512 * h + 512],
                                start=(cc == 0), stop=(cc == nch - 1))

                av_start = 3 if nch <= 4 else 4
                for c in range(nch):
                    if c + 1 < nch:
                        sc_next = emit_scores(p, tb, c + 1)
                        drain_due((bi, c))
                    else:
                        # cross-block lookahead: its qt/kt writes must be
                        # emitted first, so drain before the scores here
                        drain_due((bi, c))
                        sc_next = None
                        if bi + 1 < len(blocks):
                            ntb, np_ = blocks[bi + 1]
                            pre_sc = emit_scores(np_, ntb, 0)
                    if c == 0 and prev_norm is not None:
                        prev_bc = emit_norm2a(prev_norm)
                    d = emit_exp(p, tb, c, sc, av_queue)
                    if c == 2 and prev_norm is not None:
                        emit_norm2b(prev_norm, prev_bc)
                        prev_norm = None
                    debt[0] = min(debt[0] + d, 3500.0)
                    pump()
                    if c >= av_start:
                        flush_avs(1)
                    sc = sc_next
                flush_avs(0)
                ds = emit_norm1(tb, p, av0, av1)
                prev_norm = (tb, p, av0, av1, ds)

            # ---- tail ----
            state_tail[0] = True
            prev_bc = emit_norm2a(prev_norm)
            for tt, eb in reserved[:2]:
                emit_proj_group_direct(tt, eb)
            emit_norm2b(prev_norm, prev_bc)
            for tt, eb in reserved[2:]:
                emit_proj_group_direct(tt, eb)
            while units:
                _, _, _, fn = units.pop(0)
                fn()

    nc.compile()
    return nc


def get_nc():
    if "nc" not in _cache:
        _cache["nc"] = _build_nc()
    return _cache["nc"]


def make_in_maps(x, w_qkv, b_qkv, w_out, b_out):
    """Per-core input dicts. Core = b*2 + g."""
    x = np.asarray(x, dtype=np.float32)
    w_qkv = np.asarray(w_qkv, dtype=np.float32)
    b_qkv = np.asarray(b_qkv, dtype=np.float32)
    w_out = np.asarray(w_out, dtype=np.float32)

    wq_full, wk_full, wv_full = w_qkv[:, 0:E], w_qkv[:, E:2 * E], w_qkv[:, 2 * E:3 * E]
    bq_full, bk_full, bv_full = b_qkv[0:E], b_qkv[E:2 * E], b_qkv[2 * E:3 * E]

    idx = np.arange(128)
    tri = (idx[:, None] <= idx[None, :]).astype(np.float32)  # tri[s,t]=1 iff s<=t

    in_maps = []
    for core in range(NCORES):
        b, g = core // 2, core % 2
        h0 = g * HL
        cols = slice(h0 * D, (h0 + HL) * D)
        wq_l = wq_full[:, cols]
        wk_l = wk_full[:, cols]
        wv_l = wv_full[:, cols]
        bq_l = bq_full[cols]
        bk_l = bk_full[cols]
        bv_l = bv_full[cols]

        wqk_s = np.empty((2 * NP, 128, NE, 128), dtype=np.float16)
        for p in range(NP):
            wqk_s[2 * p] = wq_l[:, p * 128:(p + 1) * 128].reshape(NE, 128, 128).transpose(1, 0, 2)
            wqk_s[2 * p + 1] = wk_l[:, p * 128:(p + 1) * 128].reshape(NE, 128, 128).transpose(1, 0, 2)

        wv2 = np.zeros((E, VW), dtype=np.float16)
        bv2 = np.zeros((1, VW), dtype=np.float16)
        for h in range(HL):
            wv2[:, h * 65:h * 65 + 64] = wv_l[:, h * 64:(h + 1) * 64].astype(np.float16)
            bv2[0, h * 65:h * 65 + 64] = bv_l[h * 64:(h + 1) * 64].astype(np.float16)
            bv2[0, h * 65 + 64] = 1.0

        bcol = np.zeros((128, 2 * NP), dtype=np.float32)
        for p in range(NP):
            bcol[:, 2 * p] = bq_l[p * 128:(p + 1) * 128]
            bcol[:, 2 * p + 1] = bk_l[p * 128:(p + 1) * 128]

        wv2d = wv2.reshape(NE, 128, 2, VW // 2).transpose(2, 1, 0, 3)
        in_maps.append({
            "xT": np.ascontiguousarray(x[b].T.astype(np.float16)),
            "wqk": np.ascontiguousarray(wqk_s),
            "wv2d": np.ascontiguousarray(wv2d),
            "wo": np.ascontiguousarray(w_out[g * EL:(g + 1) * EL, :]).astype(np.float16),
            "rowsd": bv2,
            "bcold": bcol,
            "trid": tri,
        })
    return in_maps


def gather_output(results, b_out):
    out = np.empty((B, T, E), dtype=np.float32)
    for b in range(B):
        out[b] = (results[2 * b]["y"].astype(np.float32)
                  + results[2 * b + 1]["y"].astype(np.float32) + b_out[None, :])
    return out


def kernel(x, w_qkv, b_qkv, w_out, b_out):
    from concourse.bass_utils import run_bass_kernel_spmd

    nc = get_nc()
    in_maps = make_in_maps(x, w_qkv, b_qkv, w_out, b_out)
    r = run_bass_kernel_spmd(nc, in_maps, core_ids=list(range(NCORES)))
    return gather_output(r.results, np.asarray(b_out, dtype=np.float32))


# revision 27
# speedup vs baseline: 1.1844x; 1.0038x over previous
"""Causal self-attention (B=4, T=2048, E=1024, H=16, D=64) on 8 TRN2 NeuronCores.

Sharding: core = b*2 + g  (data parallel over batch b in 0..3, tensor parallel
over head-halves g in 0..1; 8 local heads per core, column-split QKV /
row-split out projection). Host sums the two partial out-projections per batch
and adds b_out.

Device kernel (per core), fp16 matmul operands accumulating in fp32 PSUM:
  - blocks ordered t-block-outer (tb, pair): attention for (tb, p) emits
    scoresT chunks one ahead of exp (ACT) with av accumulation behind, causal
    diagonal via [128,128] triangle multiplies on DVE.
  - a budget-driven filler pump interleaves qkv-projection / v' / out-
    projection matmul slices between attention chunks so the PE never idles
    while ACT grinds exps; earliest-deadline-first with per-chunk PE/ACT
    cycle accounting.
  - startup: PE warm-spin matmuls (HAM un-throttle) + t-block-sliced x DMAs
    so the first scores chunk lands ~8us in.
  - softmax denominators ride the v' ones-columns; normalization multiplies
    PSUM directly by a broadcast reciprocal (DVE), h1 half relocated to
    partitions 64-127 by an SBUF-to-SBUF DMA.
"""
import numpy as np

B, T, E, H, D = 4, 2048, 1024, 16, 64
HL = H // 2           # local heads per core (8)
NP = HL // 2          # head pairs per core (4)
EL = HL * D           # local attn-out width (512)
VW = HL * (D + 1)     # v' width with ones columns (520)
NCORES = 8
NB = T // 512         # t-blocks (4)
NC = T // 128         # s-chunks (16)
NE = E // 128         # e-chunks (8)
HALF = VW // 2        # 260

_cache = {}

# pump cost constants (ns, warm clock)
_ACT_FULL = 1100.0
_PE_MM512 = 512 / 2.4 + 5
_COST_QK = 2 * _PE_MM512          # 2 MMs of N=512
_COST_V = 3 * (260 / 2.4 + 5)     # 3 MMs of N=260
_COST_PROJ = 2 * _PE_MM512


def _build_nc():
    import concourse.bacc as bacc
    import concourse.mybir as mybir
    from concourse.tile import TileContext

    F32 = mybir.dt.float32
    F16 = mybir.dt.float16
    EXP = mybir.ActivationFunctionType.Exp

    nc = bacc.Bacc(None, target_bir_lowering=False)
    xT = nc.dram_tensor("xT", [E, T], F16, kind="ExternalInput")
    wqk = nc.dram_tensor("wqk", [2 * NP, 128, NE, 128], F16, kind="ExternalInput")
    wv2d = nc.dram_tensor("wv2d", [2, 128, NE, HALF], F16, kind="ExternalInput")
    wo = nc.dram_tensor("wo", [EL, E], F16, kind="ExternalInput")
    rowsd = nc.dram_tensor("rowsd", [1, VW], F16, kind="ExternalInput")   # bv2
    bcold = nc.dram_tensor("bcold", [128, 2 * NP], F32, kind="ExternalInput")
    trid = nc.dram_tensor("trid", [128, 128], F32, kind="ExternalInput")
    y = nc.dram_tensor("y", [T, E], F16, kind="ExternalOutput")

    with TileContext(nc) as tc:
        with (
            tc.tile_pool(name="const", bufs=1) as cpool,
            tc.tile_pool(name="p_keep", bufs=1) as keep,
            tc.tile_pool(name="p_st", bufs=2) as st,
            tc.tile_pool(name="p_att", bufs=6) as att,
            tc.tile_pool(name="p_nrm", bufs=2) as nrm,
            tc.tile_pool(name="p_dr", bufs=2, space="DRAM") as drp,
            tc.tile_pool(name="psA", bufs=2, space="PSUM") as psA,
        ):
            # ---- long-lived tiles ----
            xt = [keep.tile([128, T], F16, name=f"xt{e}", tag=f"xt{e}") for e in range(NE)]
            wr = {}
            for p in range(NP):
                for i, nm in enumerate(("q", "k")):
                    wr[(p, nm)] = keep.tile([128, NE, 128], F16, name=f"w{nm}{p}", tag=f"w{nm}{p}")
            wv_r = [keep.tile([128, NE, HALF], F16, name=f"wv{h_}", tag=f"wv{h_}")
                    for h_ in range(2)]
            qt = [keep.tile([128, T], F16, name=f"qt{p}", tag=f"qt{p}") for p in range(NP)]
            kt = [keep.tile([128, T], F16, name=f"kt{p}", tag=f"kt{p}") for p in range(NP)]
            vt = [keep.tile([128, VW], F16, name=f"vt{t_}", tag=f"vt{t_}") for t_ in range(NC)]
            ao = [keep.tile([128, T], F16, name=f"ao{p}", tag=f"ao{p}") for p in range(NP)]
            wo_r = keep.tile([128, NP, E], F16, name="wo_r")
            tri_sb = cpool.tile([128, 128], F32, name="tri_sb")
            bcol = cpool.tile([128, 2 * NP], F32, name="bcol")
            ones_r = cpool.tile([1, 512], F16, name="ones_r")
            bv_r = cpool.tile([1, VW], F16, name="bv_r")
            warm16 = cpool.tile([128, 128], F16, name="warm16")
            wout = cpool.tile([1, 16], F32, name="wout")

            # ---- warmup: ACT exp table + PE HAM spin ----
            nc.vector.memset(warm16, 0.5)
            nc.vector.memset(ones_r, 1.0)
            nc.scalar.activation(wout, warm16[0:1, 0:16], EXP, scale=0.125)
            wps = psA.tile([128, 512], F32, name="wps", tag="f")
            for _ in range(40):
                nc.tensor.matmul(wps[:, 0:128], warm16, warm16, start=True, stop=True)

            # ---- DMA priority order (two queues) ----
            # sync queue: pair weights + x tb0/tb1/tb2
            # gpsimd queue: v weights + x tb3 + wo
            nc.sync.dma_start(out=wr[(0, "q")], in_=wqk[0])
            nc.sync.dma_start(out=wr[(0, "k")], in_=wqk[1])
            nc.sync.dma_start(out=tri_sb, in_=trid[:, :])
            nc.sync.dma_start(out=bcol, in_=bcold[:, :])
            nc.sync.dma_start(out=bv_r, in_=rowsd[:, :])
            for e in range(NE):
                nc.sync.dma_start(out=xt[e][:, 0:512], in_=xT[e * 128:(e + 1) * 128, 0:512])
            nc.gpsimd.dma_start(out=wv_r[0], in_=wv2d[0])
            nc.gpsimd.dma_start(out=wv_r[1], in_=wv2d[1])
            for i, nm in ((1, "q"), (1, "k")):
                nc.sync.dma_start(out=wr[(i, nm)], in_=wqk[2 * i + (0 if nm == "q" else 1)])
            for e in range(NE):
                nc.gpsimd.dma_start(out=xt[e][:, 512:1024], in_=xT[e * 128:(e + 1) * 128, 512:1024])
            for i in (2, 3):
                for k, nm in enumerate(("q", "k")):
                    nc.sync.dma_start(out=wr[(i, nm)], in_=wqk[2 * i + k])
            for e in range(NE):
                nc.sync.dma_start(out=xt[e][:, 1024:1536], in_=xT[e * 128:(e + 1) * 128, 1024:1536])
            for e in range(NE):
                nc.gpsimd.dma_start(out=xt[e][:, 1536:2048], in_=xT[e * 128:(e + 1) * 128, 1536:2048])
            for p in range(NP):
                nc.gpsimd.dma_start(out=wo_r[:, p, :], in_=wo[p * 128:(p + 1) * 128, :])

            # ---- filler unit machinery ----
            units = []   # sorted list of (due, seq, cost, emit_fn)
            seq_ctr = [0]
            state_tail = [False]

            def add_unit(due, cost, fn):
                units.append((due, seq_ctr[0], cost, fn))
                seq_ctr[0] += 1

            def add_qk_units(p, nm, tb, due, on_act):
                st_ = {}
                col = 2 * p + (0 if nm == "q" else 1)
                dst = (qt if nm == "q" else kt)[p]

                def mk(i):
                    def emit():
                        if i == 0:
                            st_["ps"] = psA.tile([128, 512], F32,
                                                 name=f"fqk{p}{nm}{tb}", tag="f")
                        ps = st_["ps"]
                        for e in (2 * i, 2 * i + 1):
                            nc.tensor.matmul(
                                ps, wr[(p, nm)][:, e, :], xt[e][:, tb * 512:(tb + 1) * 512],
                                start=(e == 0), stop=(e == NE - 1))
                        if i == 3:
                            dsl = dst[:, tb * 512:(tb + 1) * 512]
                            if on_act:
                                nc.scalar.add(dsl, ps, bcol[:, col:col + 1])
                            else:
                                nc.vector.tensor_scalar_add(dsl, ps, bcol[:, col:col + 1])
                    return emit
                for i in range(4):
                    add_unit(due, _COST_QK, mk(i))

            def add_v_units(tt, h, due, on_act):
                st_ = {}

                def mk(i):
                    def emit():
                        if i == 0:
                            st_["ps"] = psA.tile([128, HALF], F32,
                                                 name=f"fv{tt}{h}", tag="f")
                        ps = st_["ps"]
                        for k in range(3 * i, 3 * i + 3):
                            if k < NE:
                                nc.tensor.matmul(
                                    ps, xt[k][:, tt * 128:(tt + 1) * 128], wv_r[h][:, k, :],
                                    start=(k == 0), stop=False)
                            else:
                                nc.tensor.matmul(
                                    ps, ones_r[:, 0:128], bv_r[:, h * HALF:(h + 1) * HALF],
                                    start=False, stop=True)
                        if i == 2:
                            dsl = vt[tt][:, h * HALF:(h + 1) * HALF]
                            if on_act:
                                nc.scalar.copy(dsl, ps)
                            else:
                                nc.vector.tensor_copy(dsl, ps)
                    return emit
                for i in range(3):
                    add_unit(due, _COST_V, mk(i))

            proj_ctr = [0]

            def add_proj_units(tt, eb):
                st_ = {}

                def mk(i):
                    def emit():
                        if i == 0:
                            st_["ps"] = psA.tile([128, 512], F32,
                                                 name=f"fp{tt}{eb}", tag="f")
                        ps = st_["ps"]
                        for pp in (2 * i, 2 * i + 1):
                            nc.tensor.matmul(
                                ps, ao[pp][:, tt * 128:(tt + 1) * 128],
                                wo_r[:, pp, eb * 512:(eb + 1) * 512],
                                start=(pp == 0), stop=(pp == NP - 1))
                        if i == 1:
                            ys = st.tile([128, 512], F16, name=f"ys{tt}{eb}",
                                         tag="ys", bufs=4)
                            proj_ctr[0] += 1
                            if state_tail[0] and proj_ctr[0] % 2 == 0:
                                nc.scalar.copy(ys, ps)
                            else:
                                nc.vector.tensor_copy(ys, ps)
                            nc.sync.dma_start(
                                out=y[tt * 128:(tt + 1) * 128, eb * 512:(eb + 1) * 512],
                                in_=ys)
                    return emit
                for i in range(2):
                    add_unit((999, proj_ctr[0]), _COST_PROJ, mk(i))

            # deadlines: qk(p,nm,tb) drained at last chunk of the previous
            # block (just before that block's cross-block scores lookahead);
            # v(tt,h) drained at chunk tt of block (tt//4, p=0).
            blocks = [(tb, p) for tb in range(NB) for p in range(NP)]
            nch_of = lambda tb: 4 * (tb + 1)
            for tb in range(NB):
                for p in range(NP):
                    bi = tb * NP + p
                    if bi == 0:
                        due = (-1, 0)
                    elif bi == 1:
                        due = (0, 2)
                    else:
                        # one extra block of slack so the cross-block scores
                        # lookahead never waits on a just-drained qk group
                        due = (bi - 2, 99)
                    for nm in ("q", "k"):
                        add_qk_units(p, nm, tb, due, on_act=(bi < 10))
            for tt in range(NC):
                for h in range(2):
                    add_v_units(tt, h, ((tt // 4) * NP, tt), on_act=(tt < 12))
            units.sort(key=lambda u: (u[0], u[1]))

            debt = [0.0]

            def drain_due(key):
                while units and units[0][0] <= key:
                    _, _, cost, fn = units.pop(0)
                    fn()
                    debt[0] = max(debt[0] - cost, -2500.0)

            def pump():
                while debt[0] > 0 and units:
                    _, _, cost, fn = units.pop(0)
                    fn()
                    debt[0] -= cost

            # ---- attention emitters ----
            def emit_scores(p, tb, c):
                j = c - 4 * tb
                lo = 128 * j if j >= 0 else 0
                sp = psA.tile([128, 1024], F32, name=f"s_{p}_{tb}_{c}", tag="s")
                nc.tensor.matmul(
                    sp[:, lo:512], kt[p][0:64, c * 128:(c + 1) * 128],
                    qt[p][0:64, tb * 512 + lo:(tb + 1) * 512],
                    start=True, stop=True, tile_position=(0, 0),
                )
                nc.tensor.matmul(
                    sp[:, 512 + lo:1024], kt[p][64:128, c * 128:(c + 1) * 128],
                    qt[p][64:128, tb * 512 + lo:(tb + 1) * 512],
                    start=True, stop=True, tile_position=(64, 0),
                )
                return sp, lo, j

            def emit_exp(p, tb, c, sc, av_queue):
                sp, lo, j = sc
                ep = att.tile([128, 1024], F16, name=f"e_{p}_{tb}_{c}", tag="ep")
                if j < 0:
                    nc.scalar.activation(ep, sp, EXP, scale=0.125)
                    act_cost = _ACT_FULL
                else:
                    spv = sp[:, :].rearrange("q (h t) -> q h t", h=2)
                    epv = ep[:, :].rearrange("q (h t) -> q h t", h=2)
                    nc.scalar.activation(epv[:, :, lo:512], spv[:, :, lo:512],
                                         EXP, scale=0.125)
                    for h in range(2):
                        nc.gpsimd.tensor_mul(
                            epv[:, h, lo:lo + 128], epv[:, h, lo:lo + 128], tri_sb)
                    act_cost = 2 * (512 - lo) * 0.833 + 250
                av_queue.append((c, ep, lo))
                pe_cost = 3 * (512 - lo) / 2.4 + 24
                return act_cost - pe_cost

            # ---- normalization (three stages, spread across the next block
            # so neither the DVE FIFO head nor the av psum slots block) ----
            def emit_norm1(tb, p, av0, av1):
                # d-rows out of PSUM (DVE; deps just met by the last av MM),
                # then bounce through DRAM for the partition broadcast
                dsb = nrm.tile([65, 1024], F32, name=f"dsb_{tb}_{p}", tag="dsb")
                nc.vector.tensor_copy(dsb[64:65, 0:512], av0[64:65, :])
                nc.vector.tensor_copy(dsb[64:65, 512:1024], av1[64:65, :])
                ds = drp.tile([1, 1024], F32, name=f"ds_{tb}_{p}", tag="ds")
                nc.gpsimd.dma_start(out=ds, in_=dsb[64:65, :])
                return ds

            def emit_norm2a(st_):
                tb, p, av0, av1, ds = st_
                bc = nrm.tile([64, 1024], F32, name=f"bc_{tb}_{p}", tag="bc")
                nc.sync.dma_start(out=bc[:, 0:512],
                                  in_=ds[0:1, 0:512].partition_broadcast(64))
                nc.sync.dma_start(out=bc[:, 512:1024],
                                  in_=ds[0:1, 512:1024].partition_broadcast(64))
                return bc

            def emit_norm2b(st_, bc):
                tb, p, av0, av1, ds = st_
                rc = nrm.tile([64, 1024], F32, name=f"rc_{tb}_{p}", tag="rc")
                nc.vector.reciprocal_approx_fast(out=rc, in_=bc)
                nc.vector.tensor_mul(ao[p][0:64, tb * 512:(tb + 1) * 512],
                                     av0[0:64, :], rc[:, 0:512])
                aot = nrm.tile([64, 512], F16, name=f"aot_{tb}_{p}", tag="aot")
                nc.vector.tensor_mul(aot, av1[0:64, :], rc[:, 512:1024])
                nc.sync.dma_start(out=ao[p][64:128, tb * 512:(tb + 1) * 512], in_=aot)
                if p == NP - 1:
                    for tt in range(4 * tb, 4 * tb + 4):
                        for eb in range(2):
                            if tb == 2 and tt >= 4 * tb + 2:
                                reserved.append((tt, eb))   # tail warm-keeper
                            else:
                                add_proj_units(tt, eb)

            reserved = []

            def emit_proj_group_direct(tt, eb):
                ps = psA.tile([128, 512], F32, name=f"rp{tt}{eb}", tag="f")
                for pp in range(NP):
                    nc.tensor.matmul(
                        ps, ao[pp][:, tt * 128:(tt + 1) * 128],
                        wo_r[:, pp, eb * 512:(eb + 1) * 512],
                        start=(pp == 0), stop=(pp == NP - 1))
                ys = st.tile([128, 512], F16, name=f"ys{tt}{eb}", tag="ys", bufs=4)
                proj_ctr[0] += 1
                if proj_ctr[0] % 2 == 0:
                    nc.scalar.copy(ys, ps)
                else:
                    nc.vector.tensor_copy(ys, ps)
                dma_eng = nc.gpsimd if proj_ctr[0] % 2 == 0 else nc.sync
                dma_eng.dma_start(
                    out=y[tt * 128:(tt + 1) * 128, eb * 512:(eb + 1) * 512], in_=ys)

            # ---- main loop ----
            pre_sc = None
            prev_norm = None
            prev_bc = None
            for bi, (tb, p) in enumerate(blocks):
                nch = nch_of(tb)
                drain_due((bi - 1, 99))
                av0 = av1 = None
                av_queue = []
                sc = pre_sc if pre_sc is not None else emit_scores(p, tb, 0)
                pre_sc = None

                def flush_avs(limit):
                    nonlocal av0, av1
                    while av_queue and len(av_queue) > limit:
                        if av0 is None:
                            av0 = psA.tile([65, 512], F32, name=f"av0_{tb}_{p}", tag="av")
                            av1 = psA.tile([65, 512], F32, name=f"av1_{tb}_{p}", tag="av")
                        cc, ep, lo = av_queue.pop(0)
                        for h, av in ((0, av0), (1, av1)):
                            vcol = 65 * (2 * p + h)
                            nc.tensor.matmul(
                                av[:, lo:512], vt[cc][:, vcol:vcol + 65],
                                ep[:, 512 * h + lo:512 * h + 512],
                                start=(cc == 0), stop=(cc == nch - 1))

                av_start = 3 if nch <= 4 else 4
                for c in range(nch):
                    if c + 1 < nch:
                        sc_next = emit_scores(p, tb, c + 1)
                        drain_due((bi, c))
                    else:
                        # cross-block lookahead: its qt/kt writes must be
                        # emitted first, so drain before the scores here
                        drain_due((bi, c))
                        sc_next = None
                        if bi + 1 < len(blocks):
                            ntb, np_ = blocks[bi + 1]
                            pre_sc = emit_scores(np_, ntb, 0)
                    if c == 0 and prev_norm is not None:
                        prev_bc = emit_norm2a(prev_norm)
                    d = emit_exp(p, tb, c, sc, av_queue)
                    if c == 2 and prev_norm is not None:
                        emit_norm2b(prev_norm, prev_bc)
                        prev_norm = None
                    debt[0] = min(debt[0] + d, 3500.0)
                    pump()
                    if c >= av_start:
                        flush_avs(1)
                    sc = sc_next
                flush_avs(0)
                ds = emit_norm1(tb, p, av0, av1)
                prev_norm = (tb, p, av0, av1, ds)

            # ---- tail ----
            state_tail[0] = True
            prev_bc = emit_norm2a(prev_norm)
            for tt, eb in reserved[:2]:
                emit_proj_group_direct(tt, eb)
            emit_norm2b(prev_norm, prev_bc)
            for tt, eb in reserved[2:]:
                emit_proj_group_direct(tt, eb)
            while units:
                _, _, _, fn = units.pop(0)
                fn()

    nc.compile()
    return nc


def get_nc():
    if "nc" not in _cache:
        _cache["nc"] = _build_nc()
    return _cache["nc"]


def make_in_maps(x, w_qkv, b_qkv, w_out, b_out):
    """Per-core input dicts. Core = b*2 + g."""
    x = np.asarray(x, dtype=np.float32)
    w_qkv = np.asarray(w_qkv, dtype=np.float32)
    b_qkv = np.asarray(b_qkv, dtype=np.float32)
    w_out = np.asarray(w_out, dtype=np.float32)

    wq_full, wk_full, wv_full = w_qkv[:, 0:E], w_qkv[:, E:2 * E], w_qkv[:, 2 * E:3 * E]
    bq_full, bk_full, bv_full = b_qkv[0:E], b_qkv[E:2 * E], b_qkv[2 * E:3 * E]

    idx = np.arange(128)
    tri = (idx[:, None] <= idx[None, :]).astype(np.float32)  # tri[s,t]=1 iff s<=t

    in_maps = []
    for core in range(NCORES):
        b, g = core // 2, core % 2
        h0 = g * HL
        cols = slice(h0 * D, (h0 + HL) * D)
        wq_l = wq_full[:, cols]
        wk_l = wk_full[:, cols]
        wv_l = wv_full[:, cols]
        bq_l = bq_full[cols]
        bk_l = bk_full[cols]
        bv_l = bv_full[cols]

        wqk_s = np.empty((2 * NP, 128, NE, 128), dtype=np.float16)
        for p in range(NP):
            wqk_s[2 * p] = wq_l[:, p * 128:(p + 1) * 128].reshape(NE, 128, 128).transpose(1, 0, 2)
            wqk_s[2 * p + 1] = wk_l[:, p * 128:(p + 1) * 128].reshape(NE, 128, 128).transpose(1, 0, 2)

        wv2 = np.zeros((E, VW), dtype=np.float16)
        bv2 = np.zeros((1, VW), dtype=np.float16)
        for h in range(HL):
            wv2[:, h * 65:h * 65 + 64] = wv_l[:, h * 64:(h + 1) * 64].astype(np.float16)
            bv2[0, h * 65:h * 65 + 64] = bv_l[h * 64:(h + 1) * 64].astype(np.float16)
            bv2[0, h * 65 + 64] = 1.0

        bcol = np.zeros((128, 2 * NP), dtype=np.float32)
        for p in range(NP):
            bcol[:, 2 * p] = bq_l[p * 128:(p + 1) * 128]
            bcol[:, 2 * p + 1] = bk_l[p * 128:(p + 1) * 128]

        wv2d = wv2.reshape(NE, 128, 2, VW // 2).transpose(2, 1, 0, 3)
        in_maps.append({
            "xT": np.ascontiguousarray(x[b].T.astype(np.float16)),
            "wqk": np.ascontiguousarray(wqk_s),
            "wv2d": np.ascontiguousarray(wv2d),
            "wo": np.ascontiguousarray(w_out[g * EL:(g + 1) * EL, :]).astype(np.float16),
            "rowsd": bv2,
            "bcold": bcol,
            "trid": tri,
        })
    return in_maps


def gather_output(results, b_out):
    out = np.empty((B, T, E), dtype=np.float32)
    for b in range(B):
        out[b] = (results[2 * b]["y"].astype(np.float32)
                  + results[2 * b + 1]["y"].astype(np.float32) + b_out[None, :])
    return out


def kernel(x, w_qkv, b_qkv, w_out, b_out):
    from concourse.bass_utils import run_bass_kernel_spmd

    nc = get_nc()
    in_maps = make_in_maps(x, w_qkv, b_qkv, w_out, b_out)
    r = run_bass_kernel_spmd(nc, in_maps, core_ids=list(range(NCORES)))
    return gather_output(r.results, np.asarray(b_out, dtype=np.float32))
